# revision 15
# baseline (speedup 1.0000x reference)
"""Trainium2 Bass kernel for nn_CascadedAttention_76836964925817.

Math: the reference module's attention machinery is dead code — softmax over a
size-1 axis is identically 1, so `context = x[0].sum(axis=0)` is a constant
and the layer reduces to the 28-dim nonlinear recurrence

    y[t] = sigmoid(Wo @ y[t-1] + Uo @ x[t-1] + c),   c = Co @ sum_t x[t],
    y[-1] = 0, x[-1] := 0.

Strategy (collective-free; every core computes the full answer redundantly —
an AllGather-based variant spent ~55us of a ~100us kernel inside the
collective waiting on peer launch skew):
  * Each core streams the FULL x as bf16 (4MB) from HBM in four 1MB slab
    DMAs on the sync HWDGE ring (1MB transfers run the SDMA engines at full
    rate; 512KB ones leave per-DMA gaps).  bf16 keeps the PE at its full
    1 col/cycle rate (fp16 runs half rate) and the end-to-end rel-norm
    error at ~2.3e-3, well inside the 2e-2 gate.
  * U = [Uo; Co] @ x.T accumulates in PSUM as a single bf16 product term,
    one 256-column window per accumulation group so the PE chases the
    stream.  Windows 0-6 fill the four upsum banks; window 7 lands in the
    spare cbboth bank so bank 3's copy/reduce can run during the stream
    and only a 256-column tail copy remains after the last byte.
  * As each bank completes, its u rows are ACT-copied into the
    column-shifted bf16 tile usb and its Co rows reduce into partial-c
    columns (vector), overlapped with the remaining stream.
  * c is totaled from the partial columns, split hi/lo into bf16 halves on
    device, and replicated across the 4 partition groups with placement
    matmuls (keeping the replication exact), then copied to SBUF as the
    activation bias.
  * Recurrence solved by 2 Jacobi fixed-point sweeps (the map is a strong
    contraction: |sigmoid'| <= 1/4, ||Wo|| ~ 0.5).  t is split into 4
    column groups of 512 stacked on partition blocks 32g..32g+27.  Sweep
    banks are pre-filled with the B term from usb via bf16 placement
    matmuls as each group's u columns become available; add_dep_helper
    pins each prefill behind a later U window so the list scheduler
    cannot hoist it into the PE stream chase and serialize the windows
    behind the scalar-engine copies.  Then
        psum += blockdiag(Wo.T) @ YA[:, 0:512]    (shifted-y storage)
        psum += shiftblk(Wo.T) @ YA[:, 512:514]   (group boundary)
    and one 128-lane sigmoid ACT with per-partition bias c writes the
    next YA; the final sweep writes the output tile in two halves so the
    first output DMA overlaps the second sigmoid.

The kernel is self-contained: shapes/sharding are hardcoded.
"""

import numpy as np

import concourse.bass as bass
import concourse.mybir as mybir
import concourse.tile as tile
from concourse import bacc
from concourse import bass_utils

F32 = mybir.dt.float32
BF16 = mybir.dt.bfloat16
AF = mybir.ActivationFunctionType

T, D, V = 2048, 1024, 28
N_CORES = 8
G = 4                      # column groups / DMA slabs
S = T // G                 # 512 columns per group
PB = 32                    # partition block stride per group (28 used + 4 pad)
PP = G * PB                # 128 partitions in the iteration phase
DCH = D // 128             # 8 contraction chunks
W2 = 64                    # padded [Uo;Co] rows: Uo 0:28, Co 32:60
K_SWEEPS = 2               # total Jacobi sweeps (incl. the B-only init sweep)
NW = 8                     # 256-col U accumulation windows
SC = T // NW               # columns per window


def build_body(nc, xt, w2t, wmm, plc, crep, yg, tc=None, reps=1):
    """Emit the program. xt:(G,128,DCH,S) x slab-major bf16;
    w2t:(128,DCH*W2) zero-padded [Uo;Co].T bf16; wmm:(PP,2,PP)
    ([.,0,.]=blockdiag(Wo.T), [.,1,.]=boundary-shift(Wo.T)) bf16;
    plc:(V,G*PP) per-group B placement bf16; crep:(W2,PP) c-replication
    placement bf16; yg:(PP,S) grouped output."""
    t = tc
    from contextlib import ExitStack
    ctx = ExitStack()
    sbp = ctx.enter_context(t.tile_pool(name="sb", bufs=1))
    pp = ctx.enter_context(t.tile_pool(name="pp", bufs=1, space="PSUM"))

    def st(shape, name, dt=F32):
        return sbp.tile(shape, dt, name=name, tag=name)

    xt_sb = st([128, G, DCH, S], "xt_sb", BF16)
    w2t_sb = st([128, DCH, W2], "w2t_sb", BF16)
    wmm_sb = st([PP, 2, PP], "wmm_sb", BF16)
    plc_sb = st([V, G * PP], "plc_sb", BF16)
    crep_sb = st([W2, PP], "crep_sb", BF16)
    cfh = st([W2, 2], "cfh", BF16)
    cfl = st([W2, 2], "cfl", BF16)
    usb = st([V, T + 1], "usb", BF16)
    ya = st([PP, S + 2], "ya", BF16)
    yfin = st([PP, S], "yfin")
    cpart = st([W2, 6], "cpart")
    cfin = st([W2, 2], "cfin")
    cbias = st([PP, 1], "cbias")
    dummy = st([1, 1], "dummy")

    upsum = pp.tile([W2, T], F32, name="upsum", tag="upsum")
    ps = [pp.tile([PP, S], F32, name=f"ps{k}", tag=f"ps{k}")
          for k in range(K_SWEEPS)]
    cbboth = pp.tile([PP, S], F32, name="cbboth", tag="cbboth")
    cb_ps = cbboth[:, 0:2]
    w7dst = cbboth[0:W2, SC:2 * SC]   # window 7's U region (spare bank)

    # Early dummy sigmoid so the ACT table load happens off the critical path.
    nc.vector.memset(dummy[:, :], 0.0)
    nc.scalar.activation(out=dummy[:, :], in_=dummy[:, :], func=AF.Sigmoid)

    # one-time constants on the gpsimd SWDGE ring (land before the sync
    # HWDGE ring's first data); w2t first since it gates the first matmuls
    nc.gpsimd.dma_start(w2t_sb[:, :, :],
                        w2t.rearrange("p (c v) -> p c v", c=DCH))
    nc.gpsimd.dma_start(wmm_sb[:, :, :], wmm)
    nc.gpsimd.dma_start(plc_sb[:, :], plc)
    nc.gpsimd.dma_start(crep_sb[:, :], crep)
    nc.vector.memset(ya[:, :].bitcast(mybir.dt.uint16), 0)
    nc.vector.memset(usb[:, 0:1].bitcast(mybir.dt.uint16), 0)
    nc.vector.memset(cfin[:, :], 0.0)
    nc.vector.memset(cfh[:, :].bitcast(mybir.dt.uint16), 0)
    nc.vector.memset(cfl[:, :].bitcast(mybir.dt.uint16), 0)

    prev_last = None
    for _rep in range(reps):
        prev_last = emit_rep(nc, t, xt, yg,
                             xt_sb, w2t_sb, wmm_sb, plc_sb,
                             crep_sb, usb, ya, yfin, cpart, cfin,
                             cfh, cfl, cbias, upsum, ps, cb_ps, w7dst,
                             prev_last)
    ctx.close()


def emit_rep(nc, t, xt, yg, xt_sb, w2t_sb, wmm_sb, plc_sb,
             crep_sb, usb, ya, yfin, cpart, cfin, cfh, cfl, cbias,
             upsum, ps, cb_ps, w7dst, prev_last=None):
    from concourse.tile_rust import add_dep_helper

    # ------- stream x: four 1MB slab DMAs on the sync HWDGE ring -------
    for q in range(G):
        d = nc.sync.dma_start(xt_sb[:, q, :, :], xt[q, :, :, :])
        if q == 0 and prev_last is not None:
            add_dep_helper(d.ins, prev_last.ins,
                           reason="serialize reps for latency measurement")

    # -------- U = [Uo;Co] @ x.T -> (64, 2048) fp32, 256-col windows -------
    def prefill_mm(k, g, stop, pin):
        m = nc.tensor.matmul(ps[k][:, :],
                             lhsT=plc_sb[:, PP * g:PP * (g + 1)],
                             rhs=usb[:, S * g:S * (g + 1)],
                             start=(g == 0), stop=stop)
        if pin is not None:
            add_dep_helper(m.ins, pin.ins,
                           reason="keep prefill out of the PE stream chase")
        return m

    def co_reduce(dst_col, src):
        nc.vector.tensor_reduce(out=cpart[32:32 + V, dst_col:dst_col + 1],
                                in_=src[32:32 + V, :],
                                axis=mybir.AxisListType.X,
                                op=mybir.AluOpType.add)

    wlast = []
    for j in range(NW):
        q, h = j // 2, j % 2
        dst = w7dst if j == NW - 1 else upsum[:, SC * j:SC * (j + 1)]
        m = None
        for c in range(DCH):
            m = nc.tensor.matmul(dst, lhsT=w2t_sb[:, c, :],
                                 rhs=xt_sb[:, q, c, SC * h:SC * (h + 1)],
                                 start=(c == 0), stop=(c == DCH - 1))
        wlast.append(m)
        if j in (1, 3, 5):
            # full-bank copy/reduce once the bank's second window stops —
            # a mid-bank read would serialize the next window behind it
            # (PSUM deps are bank-granular)
            g = j // 2
            bank = upsum[:, S * g:S * (g + 1)]
            nc.scalar.copy(usb[:, 1 + S * g:1 + S * (g + 1)], bank[0:V, :])
            co_reduce(g, bank)
        elif j == 6:
            # bank 3 holds only window 6 (window 7 went to the spare
            # bank), so its copy/reduce runs while window 7 streams
            nc.scalar.copy(usb[:, 1 + SC * 6:1 + SC * 7],
                           upsum[0:V, SC * 6:SC * 7])
            co_reduce(3, upsum[:, SC * 6:SC * 7])
        elif j == NW - 1:
            nc.scalar.copy(usb[:, 1 + SC * 7:1 + SC * 8], w7dst[0:V, :])
            co_reduce(4, w7dst)
        if j in (3, 5):
            # B prefills, pinned behind this window's last matmul so the
            # scheduler can only place them in PE idle gaps after it
            g = (j - 3) // 2
            for k in range(K_SWEEPS):
                prefill_mm(k, g, stop=False, pin=wlast[j])
    for k in range(K_SWEEPS):
        prefill_mm(k, 2, stop=False, pin=wlast[5])
    for k in range(K_SWEEPS):
        prefill_mm(k, G - 1, stop=(k == 0), pin=wlast[7])

    # ------- c path: total the per-bank Co-row partials, replicate --------
    # cfin is split hi/lo into bf16 halves so the replication matmul's
    # moving operand loses nothing (the PE truncates moving fp32 data).
    nc.vector.tensor_reduce(out=cfin[32:32 + V, 0:1],
                            in_=cpart[32:32 + V, 0:5],
                            axis=mybir.AxisListType.X, op=mybir.AluOpType.add)
    nc.vector.tensor_copy(cfh[32:32 + V, 0:1], cfin[32:32 + V, 0:1])
    nc.vector.tensor_tensor(cfl[32:32 + V, 0:1],
                            cfin[32:32 + V, 0:1], cfh[32:32 + V, 0:1],
                            mybir.AluOpType.subtract)
    nc.tensor.matmul(cb_ps[:, :], lhsT=crep_sb[32:32 + V, :],
                     rhs=cfh[32:32 + V, :], start=True, stop=False)
    nc.tensor.matmul(cb_ps[:, :], lhsT=crep_sb[32:32 + V, :],
                     rhs=cfl[32:32 + V, :], start=False, stop=True)
    nc.vector.tensor_copy(cbias[:, :], cb_ps[:, 0:1])

    # ---------------- Jacobi sweeps ----------------
    # YA[32g+v, j] stores y[512g + j - 1] for j in 1..512; col 0 and col 513
    # are permanent zeros.  ps[k] banks hold B (prefilled above); for k>0
    # the Wo.T matmuls accumulate into the still-open bank group.
    for k in range(K_SWEEPS):
        if k > 0:
            nc.tensor.matmul(ps[k][:, :], lhsT=wmm_sb[:, 0, :],
                             rhs=ya[:, 0:S], start=False, stop=False)
            nc.tensor.matmul(ps[k][:, 0:2], lhsT=wmm_sb[:, 1, :],
                             rhs=ya[:, S:S + 2], start=False, stop=True)
        if k < K_SWEEPS - 1:
            nc.scalar.activation(out=ya[:, 1:S + 1], in_=ps[k][:, :],
                                 func=AF.Sigmoid, bias=cbias[:, 0:1],
                                 scale=1.0)
        else:
            # halves, so the first output DMA overlaps the second sigmoid
            nc.scalar.activation(out=yfin[:, 0:S // 2],
                                 in_=ps[k][:, 0:S // 2],
                                 func=AF.Sigmoid, bias=cbias[:, 0:1],
                                 scale=1.0)
            nc.scalar.activation(out=yfin[:, S // 2:S],
                                 in_=ps[k][:, S // 2:S],
                                 func=AF.Sigmoid, bias=cbias[:, 0:1],
                                 scale=1.0)

    # ---------------- write grouped output ----------------
    nc.sync.dma_start(yg[:, 0:S // 2], yfin[:, 0:S // 2])
    return nc.scalar.dma_start(yg[:, S // 2:S], yfin[:, S // 2:S])


_CACHED_NC = {}


def _get_nc(reps=1):
    if reps not in _CACHED_NC:
        nc = bacc.Bacc("TRN2", target_bir_lowering=False, debug=False,
                       num_devices=N_CORES)
        xt = nc.dram_tensor("xt", [G, 128, DCH, S], BF16,
                            kind="ExternalInput")
        w2t = nc.dram_tensor("w2t", [128, DCH * W2], BF16,
                             kind="ExternalInput")
        wmm = nc.dram_tensor("wmm", [PP, 2, PP], BF16, kind="ExternalInput")
        plc = nc.dram_tensor("plc", [V, G * PP], BF16, kind="ExternalInput")
        crep = nc.dram_tensor("crep", [W2, PP], BF16, kind="ExternalInput")
        yg = nc.dram_tensor("yg", [PP, S], F32, kind="ExternalOutput")
        with tile.TileContext(nc) as t:
            build_body(nc, xt.ap(), w2t.ap(), wmm.ap(),
                       plc.ap(), crep.ap(), yg.ap(), tc=t, reps=reps)
        nc.compile()
        _CACHED_NC[reps] = nc
    return _CACHED_NC[reps]


def make_in_maps(x, Uo, Co, Wo):
    import ml_dtypes
    xb = np.ascontiguousarray(np.asarray(x, np.float32)[0])        # (T, D)
    # xt[q, p, c, tau] = bf16(x[S*q + tau, 128c + p])
    xt = np.ascontiguousarray(
        xb.T.reshape(DCH, 128, G, S).transpose(2, 1, 0, 3)
    ).astype(ml_dtypes.bfloat16)
    w2 = np.zeros((W2, D), np.float32)
    w2[0:V] = np.asarray(Uo, np.float32)
    w2[32:32 + V] = np.asarray(Co, np.float32)
    # w2t[p, (c, j)] = bf16(w2[j, 128c + p])
    w2t = np.ascontiguousarray(
        w2.T.reshape(DCH, 128, W2).transpose(1, 0, 2)
    ).astype(ml_dtypes.bfloat16).reshape(128, DCH * W2)
    wot = np.ascontiguousarray(np.asarray(Wo, np.float32).T)       # (V, V)
    wmm = np.zeros((PP, 2, PP), ml_dtypes.bfloat16)
    for g in range(G):
        wmm[PB * g:PB * g + V, 0, PB * g:PB * g + V] = wot
        if g > 0:
            wmm[PB * (g - 1):PB * (g - 1) + V, 1, PB * g:PB * g + V] = wot
    plc = np.zeros((V, G * PP), np.float32)
    for g in range(G):
        for v in range(V):
            plc[v, g * PP + PB * g + v] = 1.0
    plc = plc.astype(ml_dtypes.bfloat16)
    crep = np.zeros((W2, PP), ml_dtypes.bfloat16)
    for g in range(G):
        crep[32:32 + V, PB * g:PB * g + V] = np.eye(V, dtype=np.float32)
    in_map = {"xt": xt, "w2t": w2t, "wmm": wmm, "plc": plc, "crep": crep}
    return [in_map for _ in range(N_CORES)]


def unshard_output(yg):
    y = np.empty((T, V), np.float32)
    for g in range(G):
        y[g * S:(g + 1) * S, :] = yg[PB * g:PB * g + V, :].T
    return y[None]


def run(inputs, trace=False, reps=1, **kw):
    nc = _get_nc(reps)
    in_maps = make_in_maps(inputs["x"], inputs["Uo"], inputs["Co"],
                           inputs["Wo"])
    res = bass_utils.run_bass_kernel_spmd(
        nc, in_maps, core_ids=list(range(N_CORES)), trace=trace, **kw)
    return unshard_output(res.results[0]["yg"]), res


def kernel(**inputs):
    out, _ = run(inputs)
    return out


# revision 16
# speedup vs baseline: 1.0057x; 1.0057x over previous
"""Trainium2 Bass kernel for nn_CascadedAttention_76836964925817.

Math: the reference module's attention machinery is dead code — softmax over a
size-1 axis is identically 1, so `context = x[0].sum(axis=0)` is a constant
and the layer reduces to the 28-dim nonlinear recurrence

    y[t] = sigmoid(Wo @ y[t-1] + Uo @ x[t-1] + c),   c = Co @ sum_t x[t],
    y[-1] = 0, x[-1] := 0.

Strategy (collective-free; every core computes the full answer redundantly —
an AllGather-based variant spent ~55us of a ~100us kernel inside the
collective waiting on peer launch skew):
  * Each core streams the FULL x as bf16 (4MB) from HBM in four 1MB slab
    DMAs on the sync HWDGE ring (1MB transfers run the SDMA engines at full
    rate; 512KB ones leave per-DMA gaps).  bf16 keeps the PE at its full
    1 col/cycle rate (fp16 runs half rate) and the end-to-end rel-norm
    error at ~2.3e-3, well inside the 2e-2 gate.
  * U = [Uo; Co] @ x.T accumulates in PSUM as a single bf16 product term,
    one 256-column window per accumulation group so the PE chases the
    stream.  Windows 0-6 fill the four upsum banks; window 7 lands in the
    spare cbboth bank so bank 3's copy/reduce can run during the stream
    and only a 256-column tail copy remains after the last byte.
  * As each bank completes, its u rows are ACT-copied into the
    column-shifted bf16 tile usb and its Co rows reduce into partial-c
    columns (vector), overlapped with the remaining stream.
  * c is totaled from the partial columns, split hi/lo into bf16 halves on
    device, and replicated across the 4 partition groups with placement
    matmuls (keeping the replication exact), then copied to SBUF as the
    activation bias.
  * Recurrence solved by 2 Jacobi fixed-point sweeps (the map is a strong
    contraction: |sigmoid'| <= 1/4, ||Wo|| ~ 0.5).  t is split into 4
    column groups of 512 stacked on partition blocks 32g..32g+27.  Sweep
    banks are pre-filled with the B term from usb via bf16 placement
    matmuls as each group's u columns become available; add_dep_helper
    pins each prefill behind a later U window so the list scheduler
    cannot hoist it into the PE stream chase and serialize the windows
    behind the scalar-engine copies.  Then
        psum += blockdiag(Wo.T) @ YA[:, 0:512]    (shifted-y storage)
        psum += shiftblk(Wo.T) @ YA[:, 512:514]   (group boundary)
    and one 128-lane sigmoid ACT with per-partition bias c writes the
    next YA; the final sweep writes the output tile in two halves so the
    first output DMA overlaps the second sigmoid.

The kernel is self-contained: shapes/sharding are hardcoded.
"""

import numpy as np

import concourse.bass as bass
import concourse.mybir as mybir
import concourse.tile as tile
from concourse import bacc
from concourse import bass_utils

F32 = mybir.dt.float32
BF16 = mybir.dt.bfloat16
AF = mybir.ActivationFunctionType

T, D, V = 2048, 1024, 28
N_CORES = 8
G = 4                      # column groups / DMA slabs
S = T // G                 # 512 columns per group
PB = 32                    # partition block stride per group (28 used + 4 pad)
PP = G * PB                # 128 partitions in the iteration phase
DCH = D // 128             # 8 contraction chunks
W2 = 64                    # padded [Uo;Co] rows: Uo 0:28, Co 32:60
K_SWEEPS = 2               # total Jacobi sweeps (incl. the B-only init sweep)
NW = 8                     # 256-col U accumulation windows
SC = T // NW               # columns per window


def build_body(nc, xt, cc, yg, tc=None, reps=1):
    """Emit the program. xt:(G,128,DCH,S) x slab-major bf16; cc:(128,1408)
    packed constants bf16 — cols 0:512 [Uo;Co].T chunks, 512:768 the two
    Wo.T placement blocks, 768:1280 B placement (rows 0:28), 1280:1408 c
    replication (rows 32:60); yg:(PP,S) grouped output."""
    t = tc
    from contextlib import ExitStack
    ctx = ExitStack()
    sbp = ctx.enter_context(t.tile_pool(name="sb", bufs=1))
    pp = ctx.enter_context(t.tile_pool(name="pp", bufs=1, space="PSUM"))

    def st(shape, name, dt=F32):
        return sbp.tile(shape, dt, name=name, tag=name)

    xt_sb = st([128, G, DCH, S], "xt_sb", BF16)
    cc_sb = st([128, 1408], "cc_sb", BF16)
    w2t_sb = cc_sb[:, 0:512]          # [p, 64c+v] = w2[v, 128c+p]
    wmm_sb = cc_sb[:, 512:768]        # two 128-wide Wo.T placement blocks
    plc_sb = cc_sb[:, 768:1280]       # B placement, rows 0:28
    crep_sb = cc_sb[:, 1280:1408]     # c replication, rows 32:60
    cfh = st([W2, 2], "cfh", BF16)
    cfl = st([W2, 2], "cfl", BF16)
    usb = st([V, T + 1], "usb", BF16)
    ya = st([PP, S + 2], "ya", BF16)
    yfin = st([PP, S], "yfin")
    cpart = st([W2, 6], "cpart")
    cfin = st([W2, 2], "cfin")
    cbias = st([PP, 1], "cbias")
    dummy = st([1, 1], "dummy")

    upsum = pp.tile([W2, T], F32, name="upsum", tag="upsum")
    ps = [pp.tile([PP, S], F32, name=f"ps{k}", tag=f"ps{k}")
          for k in range(K_SWEEPS)]
    cbboth = pp.tile([PP, S], F32, name="cbboth", tag="cbboth")
    cb_ps = cbboth[:, 0:2]
    w7dst = cbboth[0:W2, SC:2 * SC]   # window 7's U region (spare bank)

    # Early dummy sigmoid so the ACT table load happens off the critical path.
    nc.vector.memset(dummy[:, :], 0.0)
    nc.scalar.activation(out=dummy[:, :], in_=dummy[:, :], func=AF.Sigmoid)

    # all one-time constants ride ONE DMA on the otherwise-idle scalar
    # HWDGE ring: the total dma_start count stays <= the 8 completion-
    # semaphore lanes, so no wait gets coarsened onto a later x slab
    nc.scalar.dma_start(cc_sb[:, :], cc)
    nc.vector.memset(ya[:, :].bitcast(mybir.dt.uint16), 0)
    nc.vector.memset(usb[:, 0:1].bitcast(mybir.dt.uint16), 0)
    nc.vector.memset(cfin[:, :], 0.0)
    nc.vector.memset(cfh[:, :].bitcast(mybir.dt.uint16), 0)
    nc.vector.memset(cfl[:, :].bitcast(mybir.dt.uint16), 0)

    prev_last = None
    for _rep in range(reps):
        prev_last = emit_rep(nc, t, xt, yg,
                             xt_sb, w2t_sb, wmm_sb, plc_sb,
                             crep_sb, usb, ya, yfin, cpart, cfin,
                             cfh, cfl, cbias, upsum, ps, cb_ps, w7dst,
                             prev_last)
    ctx.close()


def emit_rep(nc, t, xt, yg, xt_sb, w2t_sb, wmm_sb, plc_sb,
             crep_sb, usb, ya, yfin, cpart, cfin, cfh, cfl, cbias,
             upsum, ps, cb_ps, w7dst, prev_last=None):
    # (w2t_sb etc. are column views of the packed const tile)
    from concourse.tile_rust import add_dep_helper

    # ------- stream x: four 1MB slab DMAs on the sync HWDGE ring -------
    for q in range(G):
        d = nc.sync.dma_start(xt_sb[:, q, :, :], xt[q, :, :, :])
        if q == 0 and prev_last is not None:
            add_dep_helper(d.ins, prev_last.ins,
                           reason="serialize reps for latency measurement")

    # -------- U = [Uo;Co] @ x.T -> (64, 2048) fp32, 256-col windows -------
    def prefill_mm(k, g, stop, pin):
        m = nc.tensor.matmul(ps[k][:, :],
                             lhsT=plc_sb[0:V, PP * g:PP * (g + 1)],
                             rhs=usb[:, S * g:S * (g + 1)],
                             start=(g == 0), stop=stop)
        if pin is not None:
            add_dep_helper(m.ins, pin.ins,
                           reason="keep prefill out of the PE stream chase")
        return m

    def co_reduce(dst_col, src):
        nc.vector.tensor_reduce(out=cpart[32:32 + V, dst_col:dst_col + 1],
                                in_=src[32:32 + V, :],
                                axis=mybir.AxisListType.X,
                                op=mybir.AluOpType.add)

    wlast = []
    for j in range(NW):
        q, h = j // 2, j % 2
        dst = w7dst if j == NW - 1 else upsum[:, SC * j:SC * (j + 1)]
        m = None
        for c in range(DCH):
            m = nc.tensor.matmul(dst, lhsT=w2t_sb[:, 64 * c:64 * (c + 1)],
                                 rhs=xt_sb[:, q, c, SC * h:SC * (h + 1)],
                                 start=(c == 0), stop=(c == DCH - 1))
        wlast.append(m)
        if j in (1, 3, 5):
            # full-bank copy/reduce once the bank's second window stops —
            # a mid-bank read would serialize the next window behind it
            # (PSUM deps are bank-granular)
            g = j // 2
            bank = upsum[:, S * g:S * (g + 1)]
            nc.scalar.copy(usb[:, 1 + S * g:1 + S * (g + 1)], bank[0:V, :])
            co_reduce(g, bank)
        elif j == 6:
            # bank 3 holds only window 6 (window 7 went to the spare
            # bank), so its copy/reduce runs while window 7 streams
            nc.scalar.copy(usb[:, 1 + SC * 6:1 + SC * 7],
                           upsum[0:V, SC * 6:SC * 7])
            co_reduce(3, upsum[:, SC * 6:SC * 7])
        elif j == NW - 1:
            nc.scalar.copy(usb[:, 1 + SC * 7:1 + SC * 8], w7dst[0:V, :])
            co_reduce(4, w7dst)
        if j in (3, 5):
            # B prefills, pinned behind this window's last matmul so the
            # scheduler can only place them in PE idle gaps after it
            g = (j - 3) // 2
            for k in range(K_SWEEPS):
                prefill_mm(k, g, stop=False, pin=wlast[j])
    for k in range(K_SWEEPS):
        prefill_mm(k, 2, stop=False, pin=wlast[5])
    for k in range(K_SWEEPS):
        prefill_mm(k, G - 1, stop=(k == 0), pin=wlast[7])

    # ------- c path: total the per-bank Co-row partials, replicate --------
    # cfin is split hi/lo into bf16 halves so the replication matmul's
    # moving operand loses nothing (the PE truncates moving fp32 data).
    nc.vector.tensor_reduce(out=cfin[32:32 + V, 0:1],
                            in_=cpart[32:32 + V, 0:5],
                            axis=mybir.AxisListType.X, op=mybir.AluOpType.add)
    nc.vector.tensor_copy(cfh[32:32 + V, 0:1], cfin[32:32 + V, 0:1])
    nc.vector.tensor_tensor(cfl[32:32 + V, 0:1],
                            cfin[32:32 + V, 0:1], cfh[32:32 + V, 0:1],
                            mybir.AluOpType.subtract)
    nc.tensor.matmul(cb_ps[:, :], lhsT=crep_sb[32:32 + V, 0:PP],
                     rhs=cfh[32:32 + V, :], start=True, stop=False)
    nc.tensor.matmul(cb_ps[:, :], lhsT=crep_sb[32:32 + V, 0:PP],
                     rhs=cfl[32:32 + V, :], start=False, stop=True)
    nc.vector.tensor_copy(cbias[:, :], cb_ps[:, 0:1])

    # ---------------- Jacobi sweeps ----------------
    # YA[32g+v, j] stores y[512g + j - 1] for j in 1..512; col 0 and col 513
    # are permanent zeros.  ps[k] banks hold B (prefilled above); for k>0
    # the Wo.T matmuls accumulate into the still-open bank group.
    for k in range(K_SWEEPS):
        if k > 0:
            nc.tensor.matmul(ps[k][:, :], lhsT=wmm_sb[:, 0:PP],
                             rhs=ya[:, 0:S], start=False, stop=False)
            nc.tensor.matmul(ps[k][:, 0:2], lhsT=wmm_sb[:, PP:2 * PP],
                             rhs=ya[:, S:S + 2], start=False, stop=True)
        if k < K_SWEEPS - 1:
            nc.scalar.activation(out=ya[:, 1:S + 1], in_=ps[k][:, :],
                                 func=AF.Sigmoid, bias=cbias[:, 0:1],
                                 scale=1.0)
        else:
            # halves, so the first output DMA overlaps the second sigmoid
            nc.scalar.activation(out=yfin[:, 0:S // 2],
                                 in_=ps[k][:, 0:S // 2],
                                 func=AF.Sigmoid, bias=cbias[:, 0:1],
                                 scale=1.0)
            nc.scalar.activation(out=yfin[:, S // 2:S],
                                 in_=ps[k][:, S // 2:S],
                                 func=AF.Sigmoid, bias=cbias[:, 0:1],
                                 scale=1.0)

    # ---------------- write grouped output ----------------
    nc.sync.dma_start(yg[:, 0:S // 2], yfin[:, 0:S // 2])
    return nc.scalar.dma_start(yg[:, S // 2:S], yfin[:, S // 2:S])


_CACHED_NC = {}


def _get_nc(reps=1):
    if reps not in _CACHED_NC:
        nc = bacc.Bacc("TRN2", target_bir_lowering=False, debug=False,
                       num_devices=N_CORES)
        xt = nc.dram_tensor("xt", [G, 128, DCH, S], BF16,
                            kind="ExternalInput")
        cc = nc.dram_tensor("cc", [128, 1408], BF16, kind="ExternalInput")
        yg = nc.dram_tensor("yg", [PP, S], F32, kind="ExternalOutput")
        with tile.TileContext(nc) as t:
            build_body(nc, xt.ap(), cc.ap(), yg.ap(), tc=t, reps=reps)
        nc.compile()
        _CACHED_NC[reps] = nc
    return _CACHED_NC[reps]


def make_in_maps(x, Uo, Co, Wo):
    import ml_dtypes
    xb = np.ascontiguousarray(np.asarray(x, np.float32)[0])        # (T, D)
    # xt[q, p, c, tau] = bf16(x[S*q + tau, 128c + p])
    xt = np.ascontiguousarray(
        xb.T.reshape(DCH, 128, G, S).transpose(2, 1, 0, 3)
    ).astype(ml_dtypes.bfloat16)
    w2 = np.zeros((W2, D), np.float32)
    w2[0:V] = np.asarray(Uo, np.float32)
    w2[32:32 + V] = np.asarray(Co, np.float32)
    cc = np.zeros((128, 1408), np.float32)
    # cols 0:512 — w2t[p, 64c+v] = w2[v, 128c+p]
    cc[:, 0:512] = w2.T.reshape(DCH, 128, W2).transpose(1, 0, 2
                                                        ).reshape(128, 512)
    wot = np.asarray(Wo, np.float32).T                             # (V, V)
    for g in range(G):
        cc[PB * g:PB * g + V, 512 + PB * g:512 + PB * g + V] = wot
        if g > 0:
            cc[PB * (g - 1):PB * (g - 1) + V,
               640 + PB * g:640 + PB * g + V] = wot
        for v in range(V):
            cc[v, 768 + g * PP + PB * g + v] = 1.0                 # plc
        cc[32:32 + V, 1280 + PB * g:1280 + PB * g + V] = np.eye(V)  # crep
    cc = np.ascontiguousarray(cc).astype(ml_dtypes.bfloat16)
    in_map = {"xt": xt, "cc": cc}
    return [in_map for _ in range(N_CORES)]


def unshard_output(yg):
    y = np.empty((T, V), np.float32)
    for g in range(G):
        y[g * S:(g + 1) * S, :] = yg[PB * g:PB * g + V, :].T
    return y[None]


def run(inputs, trace=False, reps=1, **kw):
    nc = _get_nc(reps)
    in_maps = make_in_maps(inputs["x"], inputs["Uo"], inputs["Co"],
                           inputs["Wo"])
    res = bass_utils.run_bass_kernel_spmd(
        nc, in_maps, core_ids=list(range(N_CORES)), trace=trace, **kw)
    return unshard_output(res.results[0]["yg"]), res


def kernel(**inputs):
    out, _ = run(inputs)
    return out


# revision 17
# speedup vs baseline: 1.0682x; 1.0621x over previous
"""Trainium2 Bass kernel for nn_CascadedAttention_76836964925817.

Math: the reference module's attention machinery is dead code — softmax over a
size-1 axis is identically 1, so `context = x[0].sum(axis=0)` is a constant
and the layer reduces to the 28-dim nonlinear recurrence

    y[t] = sigmoid(Wo @ y[t-1] + Uo @ x[t-1] + c),   c = Co @ sum_t x[t],
    y[-1] = 0, x[-1] := 0.

Strategy (collective-free; every core computes the full answer redundantly —
an AllGather-based variant spent ~55us of a ~100us kernel inside the
collective waiting on peer launch skew):
  * Each core streams the FULL x as bf16 (4MB) from HBM in four 1MB slab
    DMAs on the sync HWDGE ring (1MB transfers run the SDMA engines at full
    rate; 512KB ones leave per-DMA gaps).  bf16 keeps the PE at its full
    1 col/cycle rate (fp16 runs half rate) and the end-to-end rel-norm
    error at ~2.3e-3, well inside the 2e-2 gate.
  * U = [Uo; Co] @ x.T accumulates in PSUM as a single bf16 product term,
    one 256-column window per accumulation group so the PE chases the
    stream.  Windows 0-6 fill the four upsum banks; window 7 lands in the
    spare cbboth bank so bank 3's copy/reduce can run during the stream
    and only a 256-column tail copy remains after the last byte.
  * As each bank completes, its u rows are ACT-copied into the
    column-shifted bf16 tile usb and its Co rows reduce into partial-c
    columns (vector), overlapped with the remaining stream.
  * c is totaled from the partial columns, split hi/lo into bf16 halves on
    device, and replicated across the 4 partition groups with placement
    matmuls (keeping the replication exact), then copied to SBUF as the
    activation bias.
  * Recurrence solved by 2 Jacobi fixed-point sweeps (the map is a strong
    contraction: |sigmoid'| <= 1/4, ||Wo|| ~ 0.5).  t is split into 4
    column groups of 512 stacked on partition blocks 32g..32g+27.  Sweep
    banks are pre-filled with the B term from usb via bf16 placement
    matmuls as each group's u columns become available; add_dep_helper
    pins each prefill behind a later U window so the list scheduler
    cannot hoist it into the PE stream chase and serialize the windows
    behind the scalar-engine copies.  Then
        psum += blockdiag(Wo.T) @ YA[:, 0:512]    (shifted-y storage)
        psum += shiftblk(Wo.T) @ YA[:, 512:514]   (group boundary)
    and one 128-lane sigmoid ACT with per-partition bias c writes the
    next YA; the final sweep writes the output tile in two halves so the
    first output DMA overlaps the second sigmoid.

The kernel is self-contained: shapes/sharding are hardcoded.
"""

import numpy as np

import concourse.bass as bass
import concourse.mybir as mybir
import concourse.tile as tile
from concourse import bacc
from concourse import bass_utils

F32 = mybir.dt.float32
BF16 = mybir.dt.bfloat16
AF = mybir.ActivationFunctionType

T, D, V = 2048, 1024, 28
N_CORES = 8
G = 4                      # column groups / DMA slabs
S = T // G                 # 512 columns per group
PB = 32                    # partition block stride per group (28 used + 4 pad)
PP = G * PB                # 128 partitions in the iteration phase
DCH = D // 128             # 8 contraction chunks
W2 = 64                    # padded [Uo;Co] rows: Uo 0:28, Co 32:60
K_SWEEPS = 2               # total Jacobi sweeps (incl. the B-only init sweep)
NW = 8                     # 256-col U accumulation windows
SC = T // NW               # columns per window


def build_body(nc, xta, xtb, cc, yg, tc=None, reps=1):
    """Emit the program. xta:(3,128,DCH,S) + xtb:(2,128,DCH,SC) x slabs
    bf16 (last 1MB slab split in two so the tail is not gated on the whole
    megabyte's completion semaphore); cc:(128,1408)
    packed constants bf16 — cols 0:512 [Uo;Co].T chunks, 512:768 the two
    Wo.T placement blocks, 768:1280 B placement (rows 0:28), 1280:1408 c
    replication (rows 32:60); yg:(PP,S) grouped output."""
    t = tc
    from contextlib import ExitStack
    ctx = ExitStack()
    sbp = ctx.enter_context(t.tile_pool(name="sb", bufs=1))
    pp = ctx.enter_context(t.tile_pool(name="pp", bufs=1, space="PSUM"))

    def st(shape, name, dt=F32):
        return sbp.tile(shape, dt, name=name, tag=name)

    xt_sb = st([128, G - 1, DCH, S], "xt_sb", BF16)
    xt7_sb = st([128, 2, DCH, SC], "xt7_sb", BF16)
    cc_sb = st([128, 1408], "cc_sb", BF16)
    w2t_sb = cc_sb[:, 0:512]          # [p, 64c+v] = w2[v, 128c+p]
    wmm_sb = cc_sb[:, 512:768]        # two 128-wide Wo.T placement blocks
    plc_sb = cc_sb[:, 768:1280]       # B placement, rows 0:28
    crep_sb = cc_sb[:, 1280:1408]     # c replication, rows 32:60
    cfh = st([W2, 2], "cfh", BF16)
    cfl = st([W2, 2], "cfl", BF16)
    usb = st([V, T + 1], "usb", BF16)
    ya = st([PP, S + 2], "ya", BF16)
    yfin = st([PP, S], "yfin")
    cpart = st([W2, 6], "cpart")
    cfin = st([W2, 2], "cfin")
    cbias = st([PP, 1], "cbias")
    dummy = st([1, 1], "dummy")

    up = [pp.tile([W2, S], F32, name=f"up{g}", tag=f"up{g}")
          for g in range(G)]
    ps = [pp.tile([PP, S], F32, name=f"ps{k}", tag=f"ps{k}")
          for k in range(K_SWEEPS)]
    cbboth = pp.tile([PP, S], F32, name="cbboth", tag="cbboth")
    cb_ps = cbboth[:, 0:2]
    w7dst = cbboth[0:W2, SC:2 * SC]   # window 7's U region (spare bank)

    # Early dummy sigmoid so the ACT table load happens off the critical path.
    nc.vector.memset(dummy[:, :], 0.0)
    nc.scalar.activation(out=dummy[:, :], in_=dummy[:, :], func=AF.Sigmoid)

    # all one-time constants ride ONE DMA on the otherwise-idle scalar
    # HWDGE ring: the total dma_start count stays <= the 8 completion-
    # semaphore lanes, so no wait gets coarsened onto a later x slab
    nc.scalar.dma_start(cc_sb[:, :], cc)
    nc.vector.memset(ya[:, :].bitcast(mybir.dt.uint16), 0)
    nc.vector.memset(usb[:, 0:1].bitcast(mybir.dt.uint16), 0)
    nc.vector.memset(cfin[:, :], 0.0)
    nc.vector.memset(cfh[:, :].bitcast(mybir.dt.uint16), 0)
    nc.vector.memset(cfl[:, :].bitcast(mybir.dt.uint16), 0)

    prev_last = None
    for _rep in range(reps):
        prev_last = emit_rep(nc, t, xta, xtb, yg,
                             xt_sb, xt7_sb, w2t_sb, wmm_sb, plc_sb,
                             crep_sb, usb, ya, yfin, cpart, cfin,
                             cfh, cfl, cbias, up, ps, cb_ps, w7dst,
                             prev_last)
    ctx.close()


def emit_rep(nc, t, xta, xtb, yg, xt_sb, xt7_sb, w2t_sb, wmm_sb, plc_sb,
             crep_sb, usb, ya, yfin, cpart, cfin, cfh, cfl, cbias,
             up, ps, cb_ps, w7dst, prev_last=None):
    # (w2t_sb etc. are column views of the packed const tile)
    from concourse.tile_rust import add_dep_helper

    # --- stream x: three 1MB slabs + two 512KB tail halves (sync ring) ---
    for q in range(G - 1):
        d = nc.sync.dma_start(xt_sb[:, q, :, :], xta[q, :, :, :])
        if q == 0 and prev_last is not None:
            add_dep_helper(d.ins, prev_last.ins,
                           reason="serialize reps for latency measurement")
    for h in range(2):
        nc.sync.dma_start(xt7_sb[:, h, :, :], xtb[h, :, :, :])

    # -------- U = [Uo;Co] @ x.T -> (64, 2048) fp32, 256-col windows -------
    def prefill_mm(k, g, stop, pin):
        m = nc.tensor.matmul(ps[k][:, :],
                             lhsT=plc_sb[0:V, PP * g:PP * (g + 1)],
                             rhs=usb[:, S * g:S * (g + 1)],
                             start=(g == 0), stop=stop)
        if pin is not None:
            add_dep_helper(m.ins, pin.ins,
                           reason="keep prefill out of the PE stream chase")
        return m

    def co_reduce(dst_col, src):
        nc.vector.tensor_reduce(out=cpart[32:32 + V, dst_col:dst_col + 1],
                                in_=src[32:32 + V, :],
                                axis=mybir.AxisListType.X,
                                op=mybir.AluOpType.add)

    wlast = []
    for j in range(NW):
        q, h = j // 2, j % 2
        if j == NW - 1:
            dst = w7dst
        elif j == NW - 2:
            dst = up[3][:, 0:SC]
        else:
            dst = up[q][:, SC * h:SC * (h + 1)]
        m = None
        for c in range(DCH):
            rhs = (xt7_sb[:, h, c, :] if q == G - 1
                   else xt_sb[:, q, c, SC * h:SC * (h + 1)])
            m = nc.tensor.matmul(dst, lhsT=w2t_sb[:, 64 * c:64 * (c + 1)],
                                 rhs=rhs, start=(c == 0),
                                 stop=(c == DCH - 1))
        wlast.append(m)
        if j in (1, 3, 5):
            # full-bank copy/reduce once the bank's second window stops
            # (each bank is its own tile, so the read serializes nothing)
            g = j // 2
            nc.scalar.copy(usb[:, 1 + S * g:1 + S * (g + 1)],
                           up[g][0:V, :])
            co_reduce(g, up[g])
        elif j == 6:
            # bank 3 holds only window 6 (window 7 went to the spare
            # bank), so its copy/reduce runs while window 7 streams
            nc.scalar.copy(usb[:, 1 + SC * 6:1 + SC * 7],
                           up[3][0:V, 0:SC])
            co_reduce(3, up[3][:, 0:SC])
        elif j == NW - 1:
            nc.scalar.copy(usb[:, 1 + SC * 7:1 + SC * 8], w7dst[0:V, :])
            co_reduce(4, w7dst)
        if j in (3, 5):
            # B prefills, pinned behind this window's last matmul so the
            # scheduler can only place them in PE idle gaps after it
            g = (j - 3) // 2
            for k in range(K_SWEEPS):
                prefill_mm(k, g, stop=False, pin=wlast[j])
    for k in range(K_SWEEPS):
        prefill_mm(k, 2, stop=False, pin=wlast[5])
    for k in range(K_SWEEPS):
        prefill_mm(k, G - 1, stop=(k == 0), pin=wlast[7])

    # ------- c path: total the per-bank Co-row partials, replicate --------
    # cfin is split hi/lo into bf16 halves so the replication matmul's
    # moving operand loses nothing (the PE truncates moving fp32 data).
    nc.vector.tensor_reduce(out=cfin[32:32 + V, 0:1],
                            in_=cpart[32:32 + V, 0:5],
                            axis=mybir.AxisListType.X, op=mybir.AluOpType.add)
    nc.vector.tensor_copy(cfh[32:32 + V, 0:1], cfin[32:32 + V, 0:1])
    nc.vector.tensor_tensor(cfl[32:32 + V, 0:1],
                            cfin[32:32 + V, 0:1], cfh[32:32 + V, 0:1],
                            mybir.AluOpType.subtract)
    nc.tensor.matmul(cb_ps[:, :], lhsT=crep_sb[32:32 + V, 0:PP],
                     rhs=cfh[32:32 + V, :], start=True, stop=False)
    nc.tensor.matmul(cb_ps[:, :], lhsT=crep_sb[32:32 + V, 0:PP],
                     rhs=cfl[32:32 + V, :], start=False, stop=True)
    nc.vector.tensor_copy(cbias[:, :], cb_ps[:, 0:1])

    # ---------------- Jacobi sweeps ----------------
    # YA[32g+v, j] stores y[512g + j - 1] for j in 1..512; col 0 and col 513
    # are permanent zeros.  ps[k] banks hold B (prefilled above); for k>0
    # the Wo.T matmuls accumulate into the still-open bank group.
    for k in range(K_SWEEPS):
        if k > 0:
            nc.tensor.matmul(ps[k][:, :], lhsT=wmm_sb[:, 0:PP],
                             rhs=ya[:, 0:S], start=False, stop=False)
            nc.tensor.matmul(ps[k][:, 0:2], lhsT=wmm_sb[:, PP:2 * PP],
                             rhs=ya[:, S:S + 2], start=False, stop=True)
        if k < K_SWEEPS - 1:
            nc.scalar.activation(out=ya[:, 1:S + 1], in_=ps[k][:, :],
                                 func=AF.Sigmoid, bias=cbias[:, 0:1],
                                 scale=1.0)
        else:
            # halves, so the first output DMA overlaps the second sigmoid
            nc.scalar.activation(out=yfin[:, 0:S // 2],
                                 in_=ps[k][:, 0:S // 2],
                                 func=AF.Sigmoid, bias=cbias[:, 0:1],
                                 scale=1.0)
            nc.scalar.activation(out=yfin[:, S // 2:S],
                                 in_=ps[k][:, S // 2:S],
                                 func=AF.Sigmoid, bias=cbias[:, 0:1],
                                 scale=1.0)

    # ---------------- write grouped output ----------------
    nc.sync.dma_start(yg[:, 0:S // 2], yfin[:, 0:S // 2])
    return nc.scalar.dma_start(yg[:, S // 2:S], yfin[:, S // 2:S])


_CACHED_NC = {}


def _get_nc(reps=1):
    if reps not in _CACHED_NC:
        nc = bacc.Bacc("TRN2", target_bir_lowering=False, debug=False,
                       num_devices=N_CORES)
        xta = nc.dram_tensor("xta", [G - 1, 128, DCH, S], BF16,
                             kind="ExternalInput")
        xtb = nc.dram_tensor("xtb", [2, 128, DCH, SC], BF16,
                             kind="ExternalInput")
        cc = nc.dram_tensor("cc", [128, 1408], BF16, kind="ExternalInput")
        yg = nc.dram_tensor("yg", [PP, S], F32, kind="ExternalOutput")
        with tile.TileContext(nc) as t:
            build_body(nc, xta.ap(), xtb.ap(), cc.ap(), yg.ap(),
                       tc=t, reps=reps)
        nc.compile()
        _CACHED_NC[reps] = nc
    return _CACHED_NC[reps]


def make_in_maps(x, Uo, Co, Wo):
    import ml_dtypes
    xb = np.ascontiguousarray(np.asarray(x, np.float32)[0])        # (T, D)
    # xt[q, p, c, tau] = bf16(x[S*q + tau, 128c + p]); last slab split
    xt = xb.T.reshape(DCH, 128, G, S).transpose(2, 1, 0, 3)
    xta = np.ascontiguousarray(xt[0:G - 1]).astype(ml_dtypes.bfloat16)
    xtb = np.ascontiguousarray(
        xt[G - 1].reshape(128, DCH, 2, SC).transpose(2, 0, 1, 3)
    ).astype(ml_dtypes.bfloat16)
    w2 = np.zeros((W2, D), np.float32)
    w2[0:V] = np.asarray(Uo, np.float32)
    w2[32:32 + V] = np.asarray(Co, np.float32)
    cc = np.zeros((128, 1408), np.float32)
    # cols 0:512 — w2t[p, 64c+v] = w2[v, 128c+p]
    cc[:, 0:512] = w2.T.reshape(DCH, 128, W2).transpose(1, 0, 2
                                                        ).reshape(128, 512)
    wot = np.asarray(Wo, np.float32).T                             # (V, V)
    for g in range(G):
        cc[PB * g:PB * g + V, 512 + PB * g:512 + PB * g + V] = wot
        if g > 0:
            cc[PB * (g - 1):PB * (g - 1) + V,
               640 + PB * g:640 + PB * g + V] = wot
        for v in range(V):
            cc[v, 768 + g * PP + PB * g + v] = 1.0                 # plc
        cc[32:32 + V, 1280 + PB * g:1280 + PB * g + V] = np.eye(V)  # crep
    cc = np.ascontiguousarray(cc).astype(ml_dtypes.bfloat16)
    in_map = {"xta": xta, "xtb": xtb, "cc": cc}
    return [in_map for _ in range(N_CORES)]


def unshard_output(yg):
    y = np.empty((T, V), np.float32)
    for g in range(G):
        y[g * S:(g + 1) * S, :] = yg[PB * g:PB * g + V, :].T
    return y[None]


def run(inputs, trace=False, reps=1, **kw):
    nc = _get_nc(reps)
    in_maps = make_in_maps(inputs["x"], inputs["Uo"], inputs["Co"],
                           inputs["Wo"])
    res = bass_utils.run_bass_kernel_spmd(
        nc, in_maps, core_ids=list(range(N_CORES)), trace=trace, **kw)
    return unshard_output(res.results[0]["yg"]), res


def kernel(**inputs):
    out, _ = run(inputs)
    return out
